# revision 37
# baseline (speedup 1.0000x reference)
"""MoE layer (B=4,S=2048,H=1024,F=4096,E=8,K=2) on 8 Trainium2 NeuronCores.

Strategy: expert-parallel. The gate (0.1% of FLOPs) + top-2 routing run on
host; tokens are gathered per expert and each of the 8 cores runs one
expert's dense FFN  y = relu(x@w1+b1)@w2+b2  over its routed tokens in
bf16 (fp32 PSUM accumulation). The host applies the combine weights and
scatter-adds the two expert contributions per token.

bf16 weights make BOTH w1 and w2 resident in SBUF (8.4MB each), so weight
HBM traffic is 16.8MB total instead of the ~100MB/core that an f32 w1
stream (re-fetched per token block) costs. All weight-chunk DMAs are
issued up front and the Tile dependency tracker gates each matmul on its
own chunk's arrival, so block 0 computes while the rest of the weights
stream in.

The prologue is DMA-latency-bound: w1 chunks arrive at ~2.1us cadence
while the queues spin up, so block 0 spans 640 tokens — its matmul1
phase consumes one w1 chunk per ~2.1us, exactly matching delivery, which
keeps the PE gap-free (and the HAM clock warm) through the prologue.
Block 0 runs all of matmul1 first (w1 stream only), then its matmul2 in
two halves (PSUM capacity), by which time the w2 stream has long landed.

Later blocks interleave the two matmuls at f-chunk granularity,
software-pipelined by two f so matmul2 never waits on the ACT that
produces its h input and block-boundary PSUM drains are covered:
  A(f): hT[f] = relu(w1[:,f]^T @ xgt + b1[f])   (x chunk moving)
  B(f): y[tt,hb] += hT[f,tt]^T @ w2[f, hb]      (w2 moving, 512 wide)
y accumulates token-major in PSUM and is written out token-major.
"""

import numpy as np

B, S, H, F, E, TOPK = 4, 2048, 1024, 4096, 8, 2
T = B * S
C = 2048          # per-expert device capacity: exactly 16 128-token tiles, so
#                   matmul2 pays zero tile padding. Seed-0 expert loads are
#                   1932..2182; the ~291 overflow tokens (1.8% of routed
#                   pairs) run through the exact host-side fp32 fallback
#                   below, as in standard MoE capacity-factor designs (but
#                   computed exactly instead of dropped).
TB = 384          # token sub-block (moving dim of matmul1)
T0 = 640          # block 0 size: 5 token tiles; A(f) ~2.1us per w1 chunk
BLOCKS = [(0, T0), (T0, TB), (T0 + TB, TB), (T0 + 2 * TB, TB), (1792, 256)]
NF = F // 128     # 32 F-chunks
KH = H // 128     # 8 H-chunks (contraction for matmul1)
HB = H // 512     # 2 output column halves of matmul2 (512 = one PSUM bank)

_NC_CACHE = {}


def _build_nc():
    import concourse.bacc as bacc
    import concourse.mybir as mybir
    from concourse.tile import TileContext

    f32 = mybir.dt.float32
    bf16 = mybir.dt.bfloat16
    Relu = mybir.ActivationFunctionType.Relu
    Copy = mybir.ActivationFunctionType.Copy

    nc = bacc.Bacc("TRN2", target_bir_lowering=False, debug=False, num_devices=E,
                   dynamic_dma_scratch_size=4096)
    xgt = nc.declare_dram_parameter("xgt", [H, C], bf16, isOutput=False)
    w1t = nc.declare_dram_parameter("w1t", [128, NF, KH, 128], bf16, isOutput=False)
    w2t = nc.declare_dram_parameter("w2t", [128, NF, H], bf16, isOutput=False)
    b1t = nc.declare_dram_parameter("b1t", [128, NF], f32, isOutput=False)
    out = nc.declare_dram_parameter("out", [C, H], f32, isOutput=True)     # token-major

    xgt_r = xgt.rearrange("(k p) c -> p k c", p=128)

    with TileContext(nc) as tc:
        with tc.tile_pool(name="res", bufs=1) as res_pool, \
             tc.tile_pool(name="x0p", bufs=1) as x0_pool, \
             tc.tile_pool(name="xp", bufs=2) as x_pool, \
             tc.tile_pool(name="hp", bufs=1) as h_pool, \
             tc.tile_pool(name="yp", bufs=6) as y_pool, \
             tc.tile_pool(name="p1", bufs=2, space="PSUM") as p1_pool, \
             tc.tile_pool(name="py", bufs=1, space="PSUM") as py_pool:
            # Resident across the whole kernel: full w1 + w2 (bf16, 64KB per
            # partition each) + biases. ~128KB of the 208KB partition budget.
            w1s = res_pool.tile([128, NF, KH, 128], bf16)
            w2s = res_pool.tile([128, NF, H], bf16)
            b1s = res_pool.tile([128, NF], f32)

            def load_x(b):
                # One dma_start per block: a single completion semaphore
                # instead of eight (per-DMA sem latency, not bytes, paces the
                # early phase).
                t0, tb = BLOCKS[b]
                xk = x_pool.tile([128, KH, TB], bf16, tag="xs")
                nc.sync.dma_start(out=xk[:, :, :tb], in_=xgt_r[:, :, t0:t0 + tb])
                return xk

            # Warm-up: the scalar engine's first activation pays a ~1.3us
            # ACT_TABLE_LOAD; run a dependency-free 1-column relu at t~0 so
            # the table is resident before ACT(f=0) lands on the critical
            # path (a late first ACT cascades p1-buffer-rotation stalls into
            # matmul1).
            warm = res_pool.tile([128, 2], f32)
            nc.vector.memset(warm[:, 0:1], 0.0)
            nc.scalar.activation(warm[:, 1:2], warm[:, 0:1], Relu,
                                 bias=warm[:, 0:1])

            # The early phase is paced by per-dma_start completion-semaphore
            # latency (~0.5-1us apiece while the queues spin up), not by
            # bytes — so use as FEW dma_starts as possible up front: one for
            # w1[0], ONE for all of block 0's x, one for b1, then per-chunk
            # w1 (its sems must stay ahead of the 2.1us/chunk A-phase
            # cadence) and 4-chunk-ganged w2.
            nc.sync.dma_start(out=w1s[:, 0], in_=w1t[:, 0])
            xs0 = x0_pool.tile([128, KH, T0], bf16, tag="xs0")
            # x0 in two column pieces, SMALLER one first: the first matmul is
            # gated on w1[0] + this piece (any multi-ring DMA completes no
            # earlier than ring-kick stagger + wire + receipt, so the gating
            # piece's size is what moves the first matmul). Block 0's A(f)
            # runs the matching 256-col sub-phase first.
            nc.sync.dma_start(out=xs0[:, :, TB:T0], in_=xgt_r[:, :, TB:T0])
            nc.sync.dma_start(out=xs0[:, :, 0:TB], in_=xgt_r[:, :, 0:TB])
            nc.sync.dma_start(out=b1s[:], in_=b1t[:])
            # Block 0 consumes the whole w1 stream before any w2 is touched:
            # stream all of w1, then all of w2.
            for f in range(1, NF):
                nc.sync.dma_start(out=w1s[:, f], in_=w1t[:, f])
            for f in range(0, NF, 4):
                nc.sync.dma_start(out=w2s[:, f:f + 4, :], in_=w2t[:, f:f + 4, :])

            def emit_a(f, xs, xoff, hs, hoff, sz):
                # matmul1 + relu: hs[f, hoff:hoff+sz] = relu(w1[f]^T@x + b1[f])
                p1 = p1_pool.tile([128, TB], f32, tag="p1")
                for k in range(KH):
                    nc.tensor.matmul(
                        p1[:, :sz], w1s[:, f, k, :], xs[:, k, xoff:xoff + sz],
                        start=(k == 0), stop=(k == KH - 1),
                    )
                nc.scalar.activation(hs[:, f, hoff:hoff + sz], p1[:, :sz], Relu,
                                     bias=b1s[:, f:f + 1])

            def emit_b(f, hs, pys, tts):
                # matmul2 partial for chunk f: y[tt,hb] += hs[f,tt]^T @ w2[f,hb]
                for i, tt in enumerate(tts):
                    hsf = hs[:, f, tt * 128:(tt + 1) * 128]
                    for hb in range(HB):
                        nc.tensor.matmul(
                            pys[i][hb][:, :], hsf,
                            w2s[:, f, hb * 512:(hb + 1) * 512],
                            start=(f == 0), stop=(f == NF - 1),
                        )

            def alloc_pys(bname, tts):
                return [[py_pool.tile([128, 512], f32, tag=f"py{i}_{hb}",
                                      name=f"py_{bname}_{tt}_{hb}")
                         for hb in range(HB)] for i, tt in enumerate(tts)]

            def drain(t0, pys, tts, last=False):
                # Drain PSUM on Vector and Scalar in parallel (both can read
                # PSUM; they target different banks). Keep the granularity
                # coarse: finer copies/DMAs at the kernel tail measured WORSE
                # (extra instruction + sem overhead beats the earlier issue) —
                # EXCEPT the very last tile of the kernel, whose copy is on
                # the exec-time critical path: split it across V+S in halves.
                for i, tt in enumerate(tts):
                    for hb in range(HB):
                        ys = y_pool.tile([128, 512], f32, tag="ys")
                        final = last and i == len(tts) - 1 and hb == HB - 1
                        if final:
                            nc.vector.tensor_copy(ys[:, 0:256],
                                                  pys[i][hb][:, 0:256])
                            nc.scalar.activation(ys[:, 256:512],
                                                 pys[i][hb][:, 256:512], Copy)
                        elif hb == 0:
                            nc.vector.tensor_copy(ys[:], pys[i][hb][:, :])
                        else:
                            nc.scalar.activation(ys[:], pys[i][hb][:, :], Copy)
                        nc.sync.dma_start(
                            out=out[t0 + tt * 128:t0 + (tt + 1) * 128,
                                    hb * 512:(hb + 1) * 512],
                            in_=ys[:])

            # ---- Block 0 (640 tokens): A-pass, then B in two PSUM halves.
            hs0 = h_pool.tile([128, NF, T0], bf16, tag="hs")
            xs_next = load_x(1)
            for f in range(NF):
                emit_a(f, xs0, TB, hs0, TB, T0 - TB)
                emit_a(f, xs0, 0, hs0, 0, TB)
            for tts in ((0, 1, 2), (3, 4)):
                pys = alloc_pys(f"b0h{tts[0]}", tts)
                for f in range(NF):
                    emit_b(f, hs0, pys, tts)
                drain(0, pys, tts)

            # ---- Blocks 1+: A/B interleaved, software-pipelined by two f.
            for b in range(1, len(BLOCKS)):
                t0, tb = BLOCKS[b]
                ntt = tb // 128
                xs = xs_next
                if b + 1 < len(BLOCKS):
                    xs_next = load_x(b + 1)
                hs = h_pool.tile([128, NF, TB], bf16, tag="hs")
                tts = tuple(range(ntt))
                pys = alloc_pys(f"b{b}", tts)
                for f in range(NF):
                    emit_a(f, xs, 0, hs, 0, tb)
                    if f >= 2:
                        emit_b(f - 2, hs, pys, tts)
                emit_b(NF - 2, hs, pys, tts)
                emit_b(NF - 1, hs, pys, tts)
                drain(t0, pys, tts, last=(b == len(BLOCKS) - 1))
    nc.compile()
    return nc


def _get_nc():
    if "nc" not in _NC_CACHE:
        _NC_CACHE["nc"] = _build_nc()
    return _NC_CACHE["nc"]


def _route(xf, gate_w, gate_b):
    """Top-2 gating identical to softmax+top_k+renorm (softmax is monotonic,
    and the softmax denominator cancels in the renormalization)."""
    z = xf @ gate_w + gate_b                      # [T, E] f32
    rows = np.arange(T)
    i1 = z.argmax(1)
    z2 = z.copy()
    z2[rows, i1] = -np.inf
    i2 = z2.argmax(1)
    d = np.exp((z[rows, i2] - z[rows, i1]).astype(np.float32))
    c1 = (1.0 / (1.0 + d)).astype(np.float32)
    c2 = (1.0 - c1).astype(np.float32)
    return i1, i2, c1, c2


def _prepare(xf, gate_w, gate_b, w1, b1, w2, b2):
    """Route tokens, build the per-expert device input maps (bf16) and the
    host-side scatter/overflow bookkeeping."""
    import ml_dtypes
    bf16 = ml_dtypes.bfloat16

    i1, i2, c1, c2 = _route(xf, gate_w, gate_b)

    in_maps = []
    scatter = []
    overflow = []
    for e in range(E):
        m1 = i1 == e
        m2 = i2 == e
        idx = np.concatenate([np.nonzero(m1)[0], np.nonzero(m2)[0]])
        wgt = np.concatenate([c1[m1], c2[m2]]).astype(np.float32)
        cnt = idx.size
        if cnt > C:
            # Capacity overflow (cannot happen for the fixed seed-0 inputs,
            # where the max expert load is 2182): compute the overflow rows
            # exactly on host so the result stays correct for any input.
            oidx, owgt = idx[C:], wgt[C:]
            h = np.maximum(xf[oidx] @ w1[e] + b1[e], 0.0)
            overflow.append((oidx, owgt, h @ w2[e] + b2[e]))
            idx, wgt, cnt = idx[:C], wgt[:C], C
        xg = np.zeros((C, H), np.float32)
        xg[:cnt] = xf[idx]
        xgt = np.ascontiguousarray(xg.T.astype(bf16))                       # [H, C]
        w1e = np.ascontiguousarray(
            w1[e].reshape(KH, 128, NF, 128).transpose(1, 2, 0, 3).astype(bf16))
        #                                                           [128,NF,KH,128]
        w2e = np.ascontiguousarray(
            w2[e].reshape(NF, 128, H).transpose(1, 0, 2).astype(bf16))  # [128,NF,H]
        b1e = np.ascontiguousarray(b1[e].reshape(NF, 128).T)            # [128,NF]
        in_maps.append({"xgt": xgt, "w1t": w1e, "w2t": w2e, "b1t": b1e})
        scatter.append((idx, wgt, cnt))
    return in_maps, scatter, overflow


def kernel(x, gate_w, gate_b, w1, b1, w2, b2):
    import os
    try:  # pragma: no cover - env probe
        from antenv.axon_hooks import get_axon_ntff_profile_hook  # noqa: F401
    except ImportError:
        # BASS_TRACE=1 in the environment would send run_bass_kernel_spmd
        # down the NTFF-profiling path, which hard-imports antenv.axon_hooks.
        # If that module is absent, disable tracing rather than crash.
        os.environ.setdefault("BASS_NEVER_TRACE", "1")
    from concourse.bass_utils import run_bass_kernel_spmd

    xf = np.ascontiguousarray(np.asarray(x, dtype=np.float32).reshape(T, H))
    gate_w = np.asarray(gate_w, dtype=np.float32)
    gate_b = np.asarray(gate_b, dtype=np.float32)
    w1 = np.asarray(w1, dtype=np.float32)
    b1 = np.asarray(b1, dtype=np.float32)
    w2 = np.asarray(w2, dtype=np.float32)
    b2 = np.asarray(b2, dtype=np.float32)

    in_maps, scatter, overflow = _prepare(xf, gate_w, gate_b, w1, b1, w2, b2)

    nc = _get_nc()
    res = run_bass_kernel_spmd(nc, in_maps, core_ids=list(range(E)))

    outf = np.zeros((T, H), np.float32)
    for e in range(E):
        idx, wgt, cnt = scatter[e]
        ye = res.results[e]["out"]                                          # [C, H]
        outf[idx] += (ye[:cnt] + b2[e]) * wgt[:, None]
    for oidx, owgt, oy in overflow:
        outf[oidx] += oy * owgt[:, None]
    return outf.reshape(B, S, H)


# revision 38
# speedup vs baseline: 1.0040x; 1.0040x over previous
"""MoE layer (B=4,S=2048,H=1024,F=4096,E=8,K=2) on 8 Trainium2 NeuronCores.

Strategy: expert-parallel. The gate (0.1% of FLOPs) + top-2 routing run on
host; tokens are gathered per expert and each of the 8 cores runs one
expert's dense FFN  y = relu(x@w1+b1)@w2+b2  over its routed tokens in
bf16 (fp32 PSUM accumulation). The host applies the combine weights and
scatter-adds the two expert contributions per token.

bf16 weights make BOTH w1 and w2 resident in SBUF (8.4MB each), so weight
HBM traffic is 16.8MB total instead of the ~100MB/core that an f32 w1
stream (re-fetched per token block) costs. All weight-chunk DMAs are
issued up front and the Tile dependency tracker gates each matmul on its
own chunk's arrival, so block 0 computes while the rest of the weights
stream in.

The prologue is DMA-latency-bound: w1 chunks arrive at ~2.1us cadence
while the queues spin up, so block 0 spans 640 tokens — its matmul1
phase consumes one w1 chunk per ~2.1us, exactly matching delivery, which
keeps the PE gap-free (and the HAM clock warm) through the prologue.
Block 0 runs all of matmul1 first (w1 stream only), then its matmul2 in
two halves (PSUM capacity), by which time the w2 stream has long landed.

Later blocks interleave the two matmuls at f-chunk granularity,
software-pipelined by two f so matmul2 never waits on the ACT that
produces its h input and block-boundary PSUM drains are covered:
  A(f): hT[f] = relu(w1[:,f]^T @ xgt + b1[f])   (x chunk moving)
  B(f): y[tt,hb] += hT[f,tt]^T @ w2[f, hb]      (w2 moving, 512 wide)
y accumulates token-major in PSUM and is written out token-major.
"""

import numpy as np

B, S, H, F, E, TOPK = 4, 2048, 1024, 4096, 8, 2
T = B * S
C = 2048          # per-expert device capacity: exactly 16 128-token tiles, so
#                   matmul2 pays zero tile padding. Seed-0 expert loads are
#                   1932..2182; the ~291 overflow tokens (1.8% of routed
#                   pairs) run through the exact host-side fp32 fallback
#                   below, as in standard MoE capacity-factor designs (but
#                   computed exactly instead of dropped).
TB = 384          # token sub-block (moving dim of matmul1)
T0 = 640          # block 0 size: 5 token tiles; A(f) ~2.1us per w1 chunk
BLOCKS = [(0, T0), (T0, TB), (T0 + TB, TB), (T0 + 2 * TB, TB), (1792, 256)]
NF = F // 128     # 32 F-chunks
KH = H // 128     # 8 H-chunks (contraction for matmul1)
HB = H // 512     # 2 output column halves of matmul2 (512 = one PSUM bank)

_NC_CACHE = {}


def _build_nc():
    import concourse.bacc as bacc
    import concourse.mybir as mybir
    from concourse.tile import TileContext

    f32 = mybir.dt.float32
    bf16 = mybir.dt.bfloat16
    Relu = mybir.ActivationFunctionType.Relu
    Copy = mybir.ActivationFunctionType.Copy

    nc = bacc.Bacc("TRN2", target_bir_lowering=False, debug=False, num_devices=E,
                   dynamic_dma_scratch_size=4096)
    xgt = nc.declare_dram_parameter("xgt", [H, C], bf16, isOutput=False)
    w1t = nc.declare_dram_parameter("w1t", [128, NF, KH, 128], bf16, isOutput=False)
    w2t = nc.declare_dram_parameter("w2t", [128, NF, H], bf16, isOutput=False)
    b1t = nc.declare_dram_parameter("b1t", [128, NF], f32, isOutput=False)
    out = nc.declare_dram_parameter("out", [C, H], f32, isOutput=True)     # token-major

    xgt_r = xgt.rearrange("(k p) c -> p k c", p=128)

    with TileContext(nc) as tc:
        with tc.tile_pool(name="res", bufs=1) as res_pool, \
             tc.tile_pool(name="x0p", bufs=1) as x0_pool, \
             tc.tile_pool(name="xp", bufs=2) as x_pool, \
             tc.tile_pool(name="hp", bufs=1) as h_pool, \
             tc.tile_pool(name="yp", bufs=6) as y_pool, \
             tc.tile_pool(name="p1", bufs=2, space="PSUM") as p1_pool, \
             tc.tile_pool(name="py", bufs=1, space="PSUM") as py_pool:
            # Resident across the whole kernel: full w1 + w2 (bf16, 64KB per
            # partition each) + biases. ~128KB of the 208KB partition budget.
            w1s = res_pool.tile([128, NF, KH, 128], bf16)
            w2s = res_pool.tile([128, NF, H], bf16)
            b1s = res_pool.tile([128, NF], f32)

            def load_x(b):
                # One dma_start per block: a single completion semaphore
                # instead of eight (per-DMA sem latency, not bytes, paces the
                # early phase).
                t0, tb = BLOCKS[b]
                xk = x_pool.tile([128, KH, TB], bf16, tag="xs")
                nc.sync.dma_start(out=xk[:, :, :tb], in_=xgt_r[:, :, t0:t0 + tb])
                return xk

            # Warm-up: the scalar engine's first activation pays a ~1.3us
            # ACT_TABLE_LOAD; run a dependency-free 1-column relu at t~0 so
            # the table is resident before ACT(f=0) lands on the critical
            # path (a late first ACT cascades p1-buffer-rotation stalls into
            # matmul1).
            warm = res_pool.tile([128, 2], f32)
            nc.vector.memset(warm[:, 0:1], 0.0)
            nc.scalar.activation(warm[:, 1:2], warm[:, 0:1], Relu,
                                 bias=warm[:, 0:1])

            # The early phase is paced by per-dma_start completion-semaphore
            # latency (~0.5-1us apiece while the queues spin up), not by
            # bytes — so use as FEW dma_starts as possible up front: one for
            # w1[0], ONE for all of block 0's x, one for b1, then per-chunk
            # w1 (its sems must stay ahead of the 2.1us/chunk A-phase
            # cadence) and 4-chunk-ganged w2.
            nc.sync.dma_start(out=w1s[:, 0], in_=w1t[:, 0])
            xs0 = x0_pool.tile([128, KH, T0], bf16, tag="xs0")
            # x0 in two column pieces: A(0,h0) is gated on the 786KB first
            # piece instead of the whole 1.3MB block (any multi-ring DMA
            # completes no earlier than ring-kick stagger + wire + receipt,
            # so the gating piece's size is what moves the first matmul).
            nc.sync.dma_start(out=xs0[:, :, 0:TB], in_=xgt_r[:, :, 0:TB])
            nc.sync.dma_start(out=xs0[:, :, TB:T0], in_=xgt_r[:, :, TB:T0])
            nc.sync.dma_start(out=b1s[:], in_=b1t[:])
            # Block 0 consumes the whole w1 stream before any w2 is touched:
            # stream all of w1, then all of w2.
            for f in range(1, NF):
                nc.sync.dma_start(out=w1s[:, f], in_=w1t[:, f])
            for f in range(0, NF, 4):
                nc.sync.dma_start(out=w2s[:, f:f + 4, :], in_=w2t[:, f:f + 4, :])

            def emit_a(f, xs, xoff, hs, hoff, sz):
                # matmul1 + relu: hs[f, hoff:hoff+sz] = relu(w1[f]^T@x + b1[f])
                p1 = p1_pool.tile([128, TB], f32, tag="p1")
                for k in range(KH):
                    nc.tensor.matmul(
                        p1[:, :sz], w1s[:, f, k, :], xs[:, k, xoff:xoff + sz],
                        start=(k == 0), stop=(k == KH - 1),
                    )
                nc.scalar.activation(hs[:, f, hoff:hoff + sz], p1[:, :sz], Relu,
                                     bias=b1s[:, f:f + 1])

            def emit_b(f, hs, pys, tts):
                # matmul2 partial for chunk f: y[tt,hb] += hs[f,tt]^T @ w2[f,hb]
                for i, tt in enumerate(tts):
                    hsf = hs[:, f, tt * 128:(tt + 1) * 128]
                    for hb in range(HB):
                        nc.tensor.matmul(
                            pys[i][hb][:, :], hsf,
                            w2s[:, f, hb * 512:(hb + 1) * 512],
                            start=(f == 0), stop=(f == NF - 1),
                        )

            def alloc_pys(bname, tts):
                return [[py_pool.tile([128, 512], f32, tag=f"py{i}_{hb}",
                                      name=f"py_{bname}_{tt}_{hb}")
                         for hb in range(HB)] for i, tt in enumerate(tts)]

            def drain(t0, pys, tts):
                # Drain PSUM on Vector and Scalar in parallel (both can read
                # PSUM; they target different banks). Keep the granularity
                # coarse: finer copies/DMAs at the kernel tail measured WORSE
                # (extra instruction + sem overhead beats the earlier issue).
                for i, tt in enumerate(tts):
                    for hb in range(HB):
                        ys = y_pool.tile([128, 512], f32, tag="ys")
                        if hb == 0:
                            nc.vector.tensor_copy(ys[:], pys[i][hb][:, :])
                        else:
                            nc.scalar.activation(ys[:], pys[i][hb][:, :], Copy)
                        nc.sync.dma_start(
                            out=out[t0 + tt * 128:t0 + (tt + 1) * 128,
                                    hb * 512:(hb + 1) * 512],
                            in_=ys[:])

            # ---- Block 0 (640 tokens): A-pass, then B in two PSUM halves.
            hs0 = h_pool.tile([128, NF, T0], bf16, tag="hs")
            xs_next = load_x(1)
            for f in range(NF):
                emit_a(f, xs0, 0, hs0, 0, TB)
                emit_a(f, xs0, TB, hs0, TB, T0 - TB)
            for tts in ((0, 1, 2), (3, 4)):
                pys = alloc_pys(f"b0h{tts[0]}", tts)
                for f in range(NF):
                    emit_b(f, hs0, pys, tts)
                drain(0, pys, tts)

            # ---- Blocks 1+: A/B interleaved, software-pipelined by two f.
            for b in range(1, len(BLOCKS)):
                t0, tb = BLOCKS[b]
                ntt = tb // 128
                xs = xs_next
                if b + 1 < len(BLOCKS):
                    xs_next = load_x(b + 1)
                hs = h_pool.tile([128, NF, TB], bf16, tag="hs")
                tts = tuple(range(ntt))
                pys = alloc_pys(f"b{b}", tts)
                for f in range(NF):
                    emit_a(f, xs, 0, hs, 0, tb)
                    if f >= 2:
                        emit_b(f - 2, hs, pys, tts)
                emit_b(NF - 2, hs, pys, tts)
                emit_b(NF - 1, hs, pys, tts)
                drain(t0, pys, tts)
    nc.compile()
    return nc


def _get_nc():
    if "nc" not in _NC_CACHE:
        _NC_CACHE["nc"] = _build_nc()
    return _NC_CACHE["nc"]


def _route(xf, gate_w, gate_b):
    """Top-2 gating identical to softmax+top_k+renorm (softmax is monotonic,
    and the softmax denominator cancels in the renormalization)."""
    z = xf @ gate_w + gate_b                      # [T, E] f32
    rows = np.arange(T)
    i1 = z.argmax(1)
    z2 = z.copy()
    z2[rows, i1] = -np.inf
    i2 = z2.argmax(1)
    d = np.exp((z[rows, i2] - z[rows, i1]).astype(np.float32))
    c1 = (1.0 / (1.0 + d)).astype(np.float32)
    c2 = (1.0 - c1).astype(np.float32)
    return i1, i2, c1, c2


def _prepare(xf, gate_w, gate_b, w1, b1, w2, b2):
    """Route tokens, build the per-expert device input maps (bf16) and the
    host-side scatter/overflow bookkeeping."""
    import ml_dtypes
    bf16 = ml_dtypes.bfloat16

    i1, i2, c1, c2 = _route(xf, gate_w, gate_b)

    in_maps = []
    scatter = []
    overflow = []
    for e in range(E):
        m1 = i1 == e
        m2 = i2 == e
        idx = np.concatenate([np.nonzero(m1)[0], np.nonzero(m2)[0]])
        wgt = np.concatenate([c1[m1], c2[m2]]).astype(np.float32)
        cnt = idx.size
        if cnt > C:
            # Capacity overflow (cannot happen for the fixed seed-0 inputs,
            # where the max expert load is 2182): compute the overflow rows
            # exactly on host so the result stays correct for any input.
            oidx, owgt = idx[C:], wgt[C:]
            h = np.maximum(xf[oidx] @ w1[e] + b1[e], 0.0)
            overflow.append((oidx, owgt, h @ w2[e] + b2[e]))
            idx, wgt, cnt = idx[:C], wgt[:C], C
        xg = np.zeros((C, H), np.float32)
        xg[:cnt] = xf[idx]
        xgt = np.ascontiguousarray(xg.T.astype(bf16))                       # [H, C]
        w1e = np.ascontiguousarray(
            w1[e].reshape(KH, 128, NF, 128).transpose(1, 2, 0, 3).astype(bf16))
        #                                                           [128,NF,KH,128]
        w2e = np.ascontiguousarray(
            w2[e].reshape(NF, 128, H).transpose(1, 0, 2).astype(bf16))  # [128,NF,H]
        b1e = np.ascontiguousarray(b1[e].reshape(NF, 128).T)            # [128,NF]
        in_maps.append({"xgt": xgt, "w1t": w1e, "w2t": w2e, "b1t": b1e})
        scatter.append((idx, wgt, cnt))
    return in_maps, scatter, overflow


def kernel(x, gate_w, gate_b, w1, b1, w2, b2):
    import os
    try:  # pragma: no cover - env probe
        from antenv.axon_hooks import get_axon_ntff_profile_hook  # noqa: F401
    except ImportError:
        # BASS_TRACE=1 in the environment would send run_bass_kernel_spmd
        # down the NTFF-profiling path, which hard-imports antenv.axon_hooks.
        # If that module is absent, disable tracing rather than crash.
        os.environ.setdefault("BASS_NEVER_TRACE", "1")
    from concourse.bass_utils import run_bass_kernel_spmd

    xf = np.ascontiguousarray(np.asarray(x, dtype=np.float32).reshape(T, H))
    gate_w = np.asarray(gate_w, dtype=np.float32)
    gate_b = np.asarray(gate_b, dtype=np.float32)
    w1 = np.asarray(w1, dtype=np.float32)
    b1 = np.asarray(b1, dtype=np.float32)
    w2 = np.asarray(w2, dtype=np.float32)
    b2 = np.asarray(b2, dtype=np.float32)

    in_maps, scatter, overflow = _prepare(xf, gate_w, gate_b, w1, b1, w2, b2)

    nc = _get_nc()
    res = run_bass_kernel_spmd(nc, in_maps, core_ids=list(range(E)))

    outf = np.zeros((T, H), np.float32)
    for e in range(E):
        idx, wgt, cnt = scatter[e]
        ye = res.results[e]["out"]                                          # [C, H]
        outf[idx] += (ye[:cnt] + b2[e]) * wgt[:, None]
    for oidx, owgt, oy in overflow:
        outf[oidx] += oy * owgt[:, None]
    return outf.reshape(B, S, H)
